# revision 7
# baseline (speedup 1.0000x reference)
"""Trainium2 Bass kernel for segmented attention — v3.

Key ideas vs v2:
  - TWO independent packed layouts: qT/kT pack segments whole (scores
    stay single-segment matmuls), arranged so concurrent waves use
    disjoint 32-row PE strips; the ctx/pu/Wo side SPLITS the two 84-row
    segments into 56+28 pieces so no pv/out-proj matmul rounds its PE
    tile to 128 columns (which would hog all four strips).
  - pv: all pieces of a pack tile accumulate into ONE PSUM bank at
    disjoint partition strips -> concurrent tiled matmuls.
  - biases folded into matmuls via an all-ones row appended to hsT
    (weight row 441 = bias); output bias rides on segment 0's
    denominator row of cxT (den * recip ~= 1).
  - startup DMAs spread across five engine queues; batch-1 projections
    interleaved into batch-0 attention so PE has filler while ACT
    grinds exps.
"""

import os
import math
import numpy as np
from contextlib import ExitStack

import concourse.bacc as bacc
import concourse.tile as tile
import concourse.mybir as mybir
from concourse.bass_utils import run_bass_kernel_spmd

F32 = mybir.dt.float32
BF16 = mybir.dt.bfloat16
AF = mybir.ActivationFunctionType

HID = 441
HIDA = HID + 1  # +1 ones row for bias folding
HID2 = HID + 1  # Wo free-dim pad to even
S = 1024
SH = 512
B = 16
N_CORES = 8
BPC = B // N_CORES
BOUNDS = [0, 7, 21, 49, 105, 161, 217, 273, 357, 441]
NSEG = 9
DSEG = [BOUNDS[i + 1] - BOUNDS[i] for i in range(NSEG)]
NHC = 4
HCH_IN = [(i * 128, min(128, HIDA - i * 128)) for i in range(NHC)]  # 442 rows
HCH_OUT = [(i * 128, min(128, HID - i * 128)) for i in range(NHC)]  # 441 rows
NTC = 8
NPT = 5

# ---- scores-side packing of q/k rows: whole segments ----
# seg -> (pack_tile, base). Waves pair segments with disjoint strips.
SC_PACK = {
    7: (0, 0),
    8: (1, 0),
    3: (2, 0),
    5: (2, 64),
    4: (3, 0),
    6: (3, 64),
    2: (4, 0),
    1: (4, 32),
    0: (4, 64),
}
SC_WAVES = [[7], [8], [3, 5], [4, 6], [2, 1], [0]]

# ---- ctx-side packing: pieces (name, seg, src_off, ln, pt, pb, has_den) ----
PIECES = [
    ("A7", 7, 0, 56, 0, 0, False),
    ("s5", 5, 0, 56, 0, 64, True),
    ("A8", 8, 0, 56, 1, 0, False),
    ("s6", 6, 0, 56, 1, 64, True),
    ("s3", 3, 0, 56, 2, 0, True),
    ("s4", 4, 0, 56, 2, 64, True),
    ("B7", 7, 56, 28, 3, 0, True),
    ("B8", 8, 56, 28, 3, 64, True),
    ("s2", 2, 0, 28, 3, 96, True),
    ("s1", 1, 0, 14, 4, 64, True),
    ("s0", 0, 0, 7, 4, 0, True),
]
PBYN = {p[0]: p for p in PIECES}
# col-strip base 32 is avoided everywhere: matmuls with tile_position
# (0, 32) produce garbage on this hardware (col quadrant 1 bug).
PV_TILES = [["A7", "s5"], ["A8", "s6"], ["s3", "s4"], ["B7", "B8", "s2"], ["s0", "s1"]]

# out-projection: two concurrent accumulation groups (two PSUM banks).
# Within a group every CONSECUTIVE pair overlaps PE row strips so the
# chain serializes in hardware (disjoint-strip same-bank accumulation
# would race). s0 uses ln+1 rows: its denominator row of cxT is ~1.0
# and Wo row 7 carries the output bias.
WO_G0 = ["A7", "A8", "s3", "B7", "s0"]
WO_G1 = ["s5", "B8", "s6", "s2", "s4", "s1"]
WO_ORDER = []
for i in range(max(len(WO_G0), len(WO_G1))):
    if i < len(WO_G0):
        WO_ORDER.append((WO_G0[i], 0, i == 0, i == len(WO_G0) - 1))
    if i < len(WO_G1):
        WO_ORDER.append((WO_G1[i], 1, i == 0, i == len(WO_G1) - 1))

AUG_OFF = [BOUNDS[i] + i for i in range(NSEG)]
AUG_W = HID + NSEG  # 450

_CACHE = {}


def _build():
    nc = bacc.Bacc("TRN2", target_bir_lowering=False, debug=False)

    hsT = nc.dram_tensor("hsT", [BPC, HIDA, S], BF16, kind="ExternalInput").ap()
    Wqp_d = nc.dram_tensor("Wqp", [HIDA, NPT * 128], BF16, kind="ExternalInput").ap()
    Wkp_d = nc.dram_tensor("Wkp", [HIDA, NPT * 128], BF16, kind="ExternalInput").ap()
    Wva_d = nc.dram_tensor("Wva", [HIDA, AUG_W], BF16, kind="ExternalInput").ap()
    Wop_d = nc.dram_tensor("Wop", [NPT, 128, HID2], BF16, kind="ExternalInput").ap()
    indp_d = nc.dram_tensor("indp", [NPT, NSEG, 128], BF16, kind="ExternalInput").ap()
    outT = nc.dram_tensor("outT", [BPC, HID, S], F32, kind="ExternalOutput").ap()

    with tile.TileContext(nc) as tc, ExitStack() as ctx, nc.allow_low_precision(
        reason="bf16 matmuls + bf16 softmax intermediates"
    ):
        cpool = ctx.enter_context(tc.tile_pool(name="c", bufs=1))
        hpool = ctx.enter_context(tc.tile_pool(name="h", bufs=1))
        qkpool = ctx.enter_context(tc.tile_pool(name="qk", bufs=1))
        vpool = ctx.enter_context(tc.tile_pool(name="v", bufs=1))
        epool = ctx.enter_context(tc.tile_pool(name="e", bufs=6))
        upool = ctx.enter_context(tc.tile_pool(name="u", bufs=1))
        dpool = ctx.enter_context(tc.tile_pool(name="d", bufs=2))
        cxpool = ctx.enter_context(tc.tile_pool(name="cx", bufs=1))
        opool = ctx.enter_context(tc.tile_pool(name="o", bufs=2))
        ps_sc = ctx.enter_context(tc.tile_pool(name="psc", bufs=2, space="PSUM"))
        ps_pu = ctx.enter_context(tc.tile_pool(name="ppu", bufs=2, space="PSUM"))
        ps_x = ctx.enter_context(tc.tile_pool(name="px", bufs=2, space="PSUM"))

        # ---- constants, spread across DMA queues for parallel startup ----
        Wq_sb, Wk_sb, Wv_sb = [], [], []
        for hc, (h0, hw) in enumerate(HCH_IN):
            t = cpool.tile([hw, NPT * 128], BF16, name=f"wq{hc}", tag=f"wq{hc}")
            nc.gpsimd.dma_start(out=t, in_=Wqp_d[h0 : h0 + hw, :])
            Wq_sb.append(t)
            t = cpool.tile([hw, NPT * 128], BF16, name=f"wk{hc}", tag=f"wk{hc}")
            nc.scalar.dma_start(out=t, in_=Wkp_d[h0 : h0 + hw, :])
            Wk_sb.append(t)
            t = cpool.tile([hw, AUG_W], BF16, name=f"wv{hc}", tag=f"wv{hc}")
            nc.gpsimd.dma_start(out=t, in_=Wva_d[h0 : h0 + hw, :])
            Wv_sb.append(t)
        Wo_sb, ind_sb = [], []
        for i in range(NPT):
            t = cpool.tile([128, HID2], BF16, name=f"wo{i}", tag=f"wo{i}")
            nc.scalar.dma_start(out=t, in_=Wop_d[i])
            Wo_sb.append(t)
            t = cpool.tile([NSEG, 128], BF16, name=f"ind{i}", tag=f"ind{i}")
            nc.gpsimd.dma_start(out=t, in_=indp_d[i])
            ind_sb.append(t)

        hs_all = {}

        def load_hs(b):
            hs = []
            for hc, (h0, hw) in enumerate(HCH_IN):
                t = hpool.tile([hw, S], BF16, name=f"hs{hc}", tag=f"hs{hc}", bufs=2)
                nc.sync.dma_start(out=t, in_=hsT[b, h0 : h0 + hw, :])
                hs.append(t)
            hs_all[b] = hs

        qk_all = {}

        def emit_qkproj(b, pt):
            """q and k projection for one scores pack tile of batch b."""
            if b not in qk_all:
                qk_all[b] = {"q": [None] * NPT, "k": [None] * NPT}
            hs = hs_all[b]
            for nm, W_sb in (("q", Wq_sb), ("k", Wk_sb)):
                qk = qkpool.tile(
                    [128, S], BF16, name=f"{nm}T{pt}", tag=f"{nm}T{pt}", bufs=2
                )
                for half in range(2):
                    pa = ps_x.tile([128, SH], F32, name=f"pp{nm}{pt}{half}", tag="x")
                    for hc, (h0, hw) in enumerate(HCH_IN):
                        nc.tensor.matmul(
                            pa[:],
                            W_sb[hc][:, pt * 128 : (pt + 1) * 128],
                            hs[hc][:, half * SH : (half + 1) * SH],
                            start=(hc == 0),
                            stop=(hc == NHC - 1),
                        )
                    nc.vector.tensor_copy(qk[:, half * SH : (half + 1) * SH], pa[:])
                qk_all[b][nm][pt] = qk

        va_all = {}

        def emit_vproj(b, sc):
            if b not in va_all:
                va_all[b] = [None] * NTC
            hs = hs_all[b]
            pv = ps_x.tile([128, AUG_W], F32, name=f"pv{sc}", tag="x")
            for hc, (h0, hw) in enumerate(HCH_IN):
                nc.tensor.matmul(
                    pv[:],
                    hs[hc][:, sc * 128 : (sc + 1) * 128],
                    Wv_sb[hc][:],
                    start=(hc == 0),
                    stop=(hc == NHC - 1),
                )
            va = vpool.tile([128, AUG_W], BF16, name=f"va{sc}", tag=f"va{sc}", bufs=2)
            nc.vector.tensor_copy(va[:], pv[:])
            va_all[b][sc] = va

        cx_all = {}

        def emit_attention_half(b, half, filler=None):
            """One query-half of attention for batch b. filler(i) is called
            between scores waves to interleave independent PE work."""
            qT, kT = qk_all[b]["q"], qk_all[b]["k"]
            vaug = va_all[b]
            if b not in cx_all:
                cx_all[b] = [
                    cxpool.tile([128, S], BF16, name=f"cx{b}{i}", tag=f"cxT{i}", bufs=2)
                    for i in range(NPT)
                ]
            cxT = cx_all[b]
            hsl = slice(half * SH, (half + 1) * SH)
            den9 = dpool.tile([NSEG, SH], F32, name="den9", tag="den9")

            # ---- scores + exp, wave order ----
            E = {}
            for wi, wave in enumerate(SC_WAVES):
                for seg in wave:
                    E[seg] = epool.tile([128, NTC * SH], BF16, name=f"E{seg}", tag="E")
                for t2 in range(NTC // 2):
                    pms = {
                        seg: ps_sc.tile([128, 2 * SH], F32, name=f"pm{seg}{t2}", tag="sc")
                        for seg in wave
                    }
                    # full-array keep-warm matmul: the HAM clock gate tracks
                    # PE utilization, and small-K scores alone leave it at
                    # 4/8 (1.2 GHz). Overwritten by the real k2=0 matmul.
                    nc.tensor.matmul(
                        pms[wave[0]][:, 0:128],
                        Wq_sb[0][:, 0:128],
                        hs_all[b][0][:, 0:128],
                        start=True,
                        stop=True,
                    )
                    for k2 in range(2):
                        t = 2 * t2 + k2
                        for seg in wave:
                            pt, pb = SC_PACK[seg]
                            d = DSEG[seg]
                            nc.tensor.matmul(
                                pms[seg][:, k2 * SH : (k2 + 1) * SH],
                                kT[pt][pb : pb + d, t * 128 : (t + 1) * 128],
                                qT[pt][pb : pb + d, hsl],
                                start=True,
                                stop=True,
                                tile_position=(pb, 0) if pb else None,
                            )
                    for seg in wave:
                        nc.scalar.activation(
                            E[seg][:, t2 * 2 * SH : (t2 + 1) * 2 * SH],
                            pms[seg][:],
                            AF.Exp,
                            scale=1.0 / math.sqrt(DSEG[seg]),
                        )
                if filler is not None:
                    filler(wi)

            # ---- pv per ctx pack tile ----
            u_sb = [None] * NPT
            for pt, tiles in enumerate(PV_TILES):
                pu = ps_pu.tile([128, SH], F32, name=f"pu{pt}", tag="pu")
                for t in range(NTC):
                    for pn in tiles:
                        _, seg, off, ln, _, pb, has_den = PBYN[pn]
                        a0 = AUG_OFF[seg] + off
                        w = ln + 1 if has_den else ln
                        nc.tensor.matmul(
                            pu[pb : pb + w, :],
                            vaug[t][:, a0 : a0 + w],
                            E[seg][:, t * SH : (t + 1) * SH],
                            start=(t == 0),
                            stop=(t == NTC - 1),
                            tile_position=(0, pb),
                            skip_group_check=True,
                        )
                u = upool.tile([128, SH], BF16, name=f"u{pt}", tag=f"u{pt}", bufs=2)
                nc.vector.tensor_copy(u[:], pu[:])
                u_sb[pt] = u
                for pn in tiles:
                    _, seg, off, ln, _, pb, has_den = PBYN[pn]
                    if has_den:
                        nc.gpsimd.dma_start(
                            out=den9[seg : seg + 1, :], in_=u[pb + ln : pb + ln + 1, :]
                        )

            # ---- normalize ----
            rec9 = dpool.tile([NSEG, SH], F32, name="rec9", tag="rec9")
            scr9 = dpool.tile([NSEG, SH], F32, name="scr9", tag="scr9")
            nc.vector.reciprocal_approx_accurate(rec9[:], den9[:], scratch=scr9[:])
            rec9b = dpool.tile([NSEG, SH], BF16, name="rec9b", tag="rec9b")
            nc.vector.tensor_copy(rec9b[:], rec9[:])
            for pt in range(NPT):
                recb = ps_x.tile([128, SH], F32, name=f"rb{pt}", tag="x")
                nc.tensor.matmul(
                    recb[:], ind_sb[pt][:], rec9b[:], start=True, stop=True
                )
                nc.vector.tensor_mul(cxT[pt][:, hsl], u_sb[pt][:], recb[:])

            # ---- output projection ----
            for hc, (h0, hw) in enumerate(HCH_OUT):
                po = [
                    ps_x.tile([128, SH], F32, name=f"po{hc}{g}", tag="x")
                    for g in range(2)
                ]
                nc.tensor.matmul(
                    po[0][:, 0:128],
                    Wq_sb[0][:, 0:128],
                    hs_all[b][0][:, 0:128],
                    start=True,
                    stop=True,
                )
                for pn, g, first, last in WO_ORDER:
                    _, seg, off, ln, pt, pb, _ = PBYN[pn]
                    w = ln + 1 if pn == "s0" else ln
                    nc.tensor.matmul(
                        po[g][0:hw, :],
                        Wo_sb[pt][pb : pb + w, h0 : h0 + hw],
                        cxT[pt][pb : pb + w, hsl],
                        start=first,
                        stop=last,
                        tile_position=(pb, 0) if pb else None,
                        skip_group_check=True,
                    )
                osb = opool.tile([128, SH], F32, name=f"osb{hc}", tag="osb")
                nc.vector.tensor_copy(osb[0:hw, :], po[0][0:hw, :])
                nc.vector.tensor_add(osb[0:hw, :], osb[0:hw, :], po[1][0:hw, :])
                nc.sync.dma_start(out=outT[b, h0 : h0 + hw, hsl], in_=osb[0:hw, :])

        # ================= emission schedule =================
        load_hs(0)
        for pt in range(NPT):
            emit_qkproj(0, pt)
        for sc in range(NTC):
            emit_vproj(0, sc)
        load_hs(1)

        def b1_proj_filler(wi):
            if wi < NPT:
                emit_qkproj(1, wi)

        emit_attention_half(0, 0, filler=b1_proj_filler)

        def b1_vproj_filler(wi):
            if wi < 4:
                emit_vproj(1, 2 * wi)
                emit_vproj(1, 2 * wi + 1)

        emit_attention_half(0, 1, filler=b1_vproj_filler)
        emit_attention_half(1, 0)
        emit_attention_half(1, 1)

    nc.compile()
    return nc


import ml_dtypes

BF16NP = ml_dtypes.bfloat16


def _prep_core_inputs(hidden_states, Wq, bq, Wk, bk, Wv, bv, Wo, bo):
    """Host-side layout prep (transpose/reorder/pad only, no math)."""
    f32 = np.float32
    hs = np.ascontiguousarray(hidden_states.astype(f32, copy=False))
    Wq = np.asarray(Wq, dtype=f32)
    Wk = np.asarray(Wk, dtype=f32)
    Wv = np.asarray(Wv, dtype=f32)
    Wo = np.asarray(Wo, dtype=f32)
    bq = np.asarray(bq, dtype=f32)
    bk = np.asarray(bk, dtype=f32)
    bv = np.asarray(bv, dtype=f32)
    bo = np.asarray(bo, dtype=f32)

    # scores-side q/k packing (whole segments)
    Wqp = np.zeros((HIDA, NPT * 128), dtype=f32)
    Wkp = np.zeros((HIDA, NPT * 128), dtype=f32)
    for seg, (pt, pb) in SC_PACK.items():
        g0, d = BOUNDS[seg], DSEG[seg]
        Wqp[:HID, pt * 128 + pb : pt * 128 + pb + d] = Wq[:, g0 : g0 + d]
        Wqp[HID, pt * 128 + pb : pt * 128 + pb + d] = bq[g0 : g0 + d]
        Wkp[:HID, pt * 128 + pb : pt * 128 + pb + d] = Wk[:, g0 : g0 + d]
        Wkp[HID, pt * 128 + pb : pt * 128 + pb + d] = bk[g0 : g0 + d]

    # ctx-side packing (split pieces)
    Wop = np.zeros((NPT, 128, HID2), dtype=BF16NP)
    indp = np.zeros((NPT, NSEG, 128), dtype=BF16NP)
    for pn, seg, off, ln, pt, pb, has_den in PIECES:
        g0 = BOUNDS[seg] + off
        Wop[pt, pb : pb + ln, :HID] = Wo[g0 : g0 + ln, :].astype(BF16NP)
        indp[pt, seg, pb : pb + ln + (1 if has_den else 0)] = 1.0
    Wop[4, 7, :HID] = bo.astype(BF16NP)  # rides on cxT's ~1.0 denom row

    Wva = np.zeros((HIDA, AUG_W), dtype=f32)
    for sg in range(NSEG):
        s0, s1 = BOUNDS[sg], BOUNDS[sg + 1]
        a0 = AUG_OFF[sg]
        Wva[:HID, a0 : a0 + (s1 - s0)] = Wv[:, s0:s1]
        Wva[HID, a0 : a0 + (s1 - s0)] = bv[s0:s1]
        Wva[HID, a0 + (s1 - s0)] = 1.0  # ones column for the denominator

    shared = {
        "Wqp": Wqp.astype(BF16NP),
        "Wkp": Wkp.astype(BF16NP),
        "Wva": Wva.astype(BF16NP),
        "Wop": Wop,
        "indp": indp,
    }
    in_maps = []
    for c in range(N_CORES):
        shard = hs[c * BPC : (c + 1) * BPC]
        hsA = np.ones((BPC, HIDA, S), dtype=BF16NP)
        hsA[:, :HID, :] = shard.transpose(0, 2, 1).astype(BF16NP)
        m = dict(shared)
        m["hsT"] = hsA
        in_maps.append(m)
    return in_maps


LAST_RESULTS = None


def kernel(hidden_states, Wq, bq, Wk, bk, Wv, bv, Wo, bo):
    global LAST_RESULTS
    if "nc" not in _CACHE:
        _CACHE["nc"] = _build()
    nc = _CACHE["nc"]
    in_maps = _prep_core_inputs(hidden_states, Wq, bq, Wk, bk, Wv, bv, Wo, bo)
    kwargs = {}
    if os.environ.get("KERNEL_TRACE") == "1":
        kwargs["trace"] = True
        td = os.environ.get("KERNEL_TRACE_DIR")
        if td:
            kwargs["tmpdir"] = td
    res = run_bass_kernel_spmd(nc, in_maps, core_ids=list(range(N_CORES)), **kwargs)
    LAST_RESULTS = res
    out = np.empty((B, S, HID), dtype=np.float32)
    for c in range(N_CORES):
        out[c * BPC : (c + 1) * BPC] = res.results[c]["outT"].transpose(0, 2, 1)
    return out
